# revision 36
# baseline (speedup 1.0000x reference)
"""Category-specific linear (MoE-routing style) Trainium2 Bass kernel.

Computes out[n] = x[n] @ W[cat_ids[n]] + b[cat_ids[n]] for
x: [N, M, D_IN] f32, cat_ids: [N] int64, W: [C, D_IN, D_H] f32, b: [C, D_H] f32.

Strategy (8-core SPMD, full inputs in / full output out):
  Host: stable-sort samples by category, split into 8 equal shards of
  N/8 samples (perfect load balance).  Within a shard, each category is a
  contiguous run; runs are padded to whole 128-row tiles (8 samples) so the
  device program is fully static.  x rows are pre-transposed on the host
  (fp32 has no DMA-transpose path on TRN2) into [2, 128, NT*128] so the
  contraction dim lands on SBUF partitions.  Each core also gets a small
  deduplicated weight table (its <=KMAX distinct categories) and a per-tile
  weight-slot index.
  Device (v2 layout, tuned from perfetto trace of v1):
    - x lives in ONE resident SBUF buffer, loaded by 6 large piece-DMAs on
      the Sync HWDGE ring (range-based Tile deps give fine-grained readiness).
    - W table in SBUF (chunk-major); per-tile weight slot loaded into PE
      registers (values_load), matmul moving operand dynamically sliced.
    - PSUM groups of 8 tiles; within a group all chunk-0 matmuls run first,
      then all chunk-1 (so the first group only waits on W chunk 0).
    - PSUM->SBUF copies are split per group between Scalar (ACT) and Vector
      (DVE) engines - v1 had all copies on DVE and it was the steady-state
      bottleneck (100% busy).
    - Output staged in one resident SBUF buffer; stores issued per group on
      the GPSIMD (SWDGE) ring so they never contend with load issue or the
      copy engines.
"""

import os
import sys

import numpy as np

for _p in ("/opt/trn_rl_repo",):
    if os.path.isdir(_p) and _p not in sys.path:
        sys.path.insert(0, _p)

import concourse.bass as bass  # noqa: E402
import concourse.mybir as mybir  # noqa: E402
import concourse.tile as tile  # noqa: E402
from concourse import bacc  # noqa: E402
from concourse.bass import ds  # noqa: E402
from concourse.bass_utils import run_bass_kernel_spmd  # noqa: E402

NCORES = 8
P = 128  # SBUF partitions / rows per tile
D_IN = 256  # contraction dim (2 chunks of 128)
D_H = 256  # output dim
ROWS_PER_SAMPLE = 16
SPT = P // ROWS_PER_SAMPLE  # samples per tile = 8
TB = 8  # tile-count quantum (NT is padded to a multiple of this)
TBI = 16  # tiles per index-register load

# filled by kernel() for test harness introspection
last_results = None


def _pack(x, cat_ids, W):
    """Host-side routing: sort, shard, pad, transpose, dedup weights.

    Returns (in_maps, scatter_info, NT, KMAX).
    scatter_info[k] = (sample_ids_per_padded_slot [NT*SPT] int64, valid mask)
    """
    N, M, Din = x.shape
    assert M == ROWS_PER_SAMPLE and Din == D_IN
    assert N % NCORES == 0

    cat = np.asarray(cat_ids).astype(np.int64).ravel()
    order = np.argsort(cat, kind="stable")
    cats_sorted = cat[order]

    # global category runs over the sorted sample list
    bounds = np.flatnonzero(np.diff(cats_sorted)) + 1
    seg_starts = np.concatenate([[0], bounds])
    seg_ends = np.concatenate([bounds, [N]])
    segments = [
        (int(cats_sorted[s]), int(s), int(e))
        for s, e in zip(seg_starts, seg_ends)
    ]

    def pack(T):
        """Greedy-pack category runs into cores of <= T tiles each.

        A run cut mid-category always cuts at an SPT-sample multiple, so
        cuts cost no padding; only each core-local run tail pads to a tile.
        Returns (padded_ids, tile_cats) per core or None if > NCORES cores.
        """
        cores = []
        cur_ids, cur_tcats, used = [], [], 0
        rem = list(segments)
        i = 0

        def close():
            nonlocal cur_ids, cur_tcats, used
            cores.append((cur_ids, cur_tcats))
            cur_ids, cur_tcats, used = [], [], 0

        while i < len(rem):
            c, s, e = rem[i]
            n = e - s
            tiles_need = (n + SPT - 1) // SPT
            avail = T - used
            if avail >= tiles_need:
                npad = (-n) % SPT
                cur_ids.append(order[s:e])
                if npad:
                    cur_ids.append(np.full(npad, -1, np.int64))
                cur_tcats.extend([c] * tiles_need)
                used += tiles_need
                i += 1
            elif avail >= 1:
                take = avail * SPT  # n > take since tiles_need > avail
                cur_ids.append(order[s : s + take])
                cur_tcats.extend([c] * avail)
                used = T
                rem[i] = (c, s + take, e)
                close()
            else:
                close()
            if len(cores) > NCORES:
                return None
        if cur_tcats:
            close()
        if len(cores) > NCORES:
            return None
        while len(cores) < NCORES:
            cores.append(([], []))
        return cores

    lo, hi = (N // NCORES) // SPT, ((N // NCORES) // SPT) * 2 + 16
    while lo < hi:
        mid = (lo + hi) // 2
        if pack(mid) is not None:
            hi = mid
        else:
            lo = mid + 1
    NT = ((lo + 3) // 4) * 4  # multiple of 4
    cores = pack(NT)
    assert cores is not None

    # pad every core to NT tiles (pad with the LAST category so the slot
    # sequence stays monotone / run-contiguous)
    padded_ids = []
    tile_cats = []
    for k in range(NCORES):
        ids_parts, tcats = cores[k]
        n_have = len(tcats)
        extra = NT - n_have
        if extra:
            fill_cat = tcats[-1] if tcats else 0
            tcats = tcats + [fill_cat] * extra
            ids_parts = ids_parts + [np.full(extra * SPT, -1, np.int64)]
        padded_ids.append(
            np.concatenate(ids_parts)
            if ids_parts
            else np.full(NT * SPT, -1, np.int64)
        )
        tile_cats.append(tcats)

    # per-core weight dedup
    uniq_list = []
    for k in range(NCORES):
        seen = dict()
        for c in tile_cats[k]:
            if c not in seen:
                seen[c] = len(seen)
        uniq_list.append(seen)
    KMAX = max(len(u) for u in uniq_list)

    np_in = _np_in_dtype()
    in_maps = []
    scatter = []
    for k in range(NCORES):
        ids = padded_ids[k]
        valid = ids >= 0
        # gather + zero-pad x rows: [NT*SPT, M, Din]
        Xr = np.zeros((NT * SPT, M, Din), np.float32)
        Xr[valid] = x[ids[valid]]
        # transpose to [Din, NT*P] then chunk the contraction dim
        xT = np.ascontiguousarray(
            Xr.reshape(NT * P, Din).T.astype(np_in)
        ).reshape(2, P, NT * P)

        seen = uniq_list[k]
        w_ids = list(seen.keys())
        w_ids += [w_ids[0]] * (KMAX - len(w_ids))
        Wp = W[np.asarray(w_ids, np.int64)]  # [KMAX, Din, D_H]
        Wl = np.ascontiguousarray(
            Wp.reshape(KMAX, 2, P, D_H).transpose(2, 1, 0, 3).astype(np_in)
        )  # [P, 2, KMAX, D_H]

        # pre-scaled by D_H so the device-side dynamic slice lands on a
        # stride-1 dim (no register multiply chain per matmul)
        widx = np.asarray(
            [seen[c] * D_H for c in tile_cats[k]], np.int32
        )[None, :]

        in_maps.append({"xT": xT, "Wl": Wl, "widx": widx})
        scatter.append((ids, valid))

    return in_maps, scatter, NT, KMAX


def _dt_mode():
    return os.environ.get("CSL_DT_MODE", "f16")


def _out_mode():
    return os.environ.get("CSL_OUT_DT", "f16")


def _np_in_dtype():
    import ml_dtypes

    return {
        "f16": np.float16,
        "bf16": ml_dtypes.bfloat16,
        "f32r": np.float32,
        "f32": np.float32,
    }[_dt_mode()]


def _mm_dt():
    return {
        "f16": mybir.dt.float16,
        "bf16": mybir.dt.bfloat16,
        "f32r": mybir.dt.float32r,
        "f32": mybir.dt.float32,
    }[_dt_mode()]


def _groups(NT):
    """Psum/copy group boundaries: 4-tile groups (fine granularity keeps the
    PE dense and copies early; any remainder lands in the last group)."""
    gs = []
    t = 0
    while t < NT:
        g = min(4, NT - t)
        gs.append((t, g))
        t += g
    return gs


def _build(NT, KMAX):
    """Build the SPMD device program for NT tiles and KMAX weight slots."""
    mm_dt = _mm_dt()
    out_dt = mybir.dt.float32 if _out_mode() == "f32" else mybir.dt.float16
    f32 = mybir.dt.float32
    i32 = mybir.dt.int32
    WB = mybir.dt.size(mm_dt)  # bytes per W element
    static_idx = os.environ.get("CSL_STATIC", "0") == "1"

    nc = bacc.Bacc(
        "TRN2",
        target_bir_lowering=False,
        debug=False,
        enable_asserts=False,
        num_devices=NCORES,
    )
    NTR = NT * P
    xT_d = nc.dram_tensor("xT", [2, P, NTR], mm_dt, kind="ExternalInput").ap()
    W_d = nc.dram_tensor("Wl", [P, 2, KMAX, D_H], mm_dt, kind="ExternalInput").ap()
    wi_d = nc.dram_tensor("widx", [1, NT], i32, kind="ExternalInput").ap()
    # partition-major output layout: fully contiguous per-partition stores;
    # the host untransposes when scattering back
    out_d = nc.dram_tensor("out", [P, NT, D_H], out_dt, kind="ExternalOutput").ap()

    groups = _groups(NT)
    # x piece boundaries (in tiles) per chunk: small head then ~4 even pieces
    # (finer pieces = earlier per-range DMA-completion sems for the matmuls)
    def _pieces(NT):
        ps = [(0, min(8, NT))]
        rest = NT - ps[0][1]
        a = ps[0][1]
        nsplit = max(1, min(4, rest // 12))
        for i in range(nsplit):
            n = rest // nsplit + (1 if i < rest % nsplit else 0)
            if n:
                ps.append((a, n))
                a += n
        return ps

    pieces = _pieces(NT)

    with tile.TileContext(nc) as tc:
        with (
            tc.tile_pool(name="wpool", bufs=1) as wpool,
            tc.tile_pool(name="xpool", bufs=1) as xpool,
            tc.tile_pool(name="opool", bufs=1) as opool,
            tc.tile_pool(name="psum", bufs=4, space="PSUM") as psum_pool,
        ):
            # --- issue order matters: widx first (tiny, unblocks the PE
            # values_load), then W chunk 0, then the first x pieces.
            wi_sb = wpool.tile([1, NT], i32)
            nc.sync.dma_start(wi_sb[:], wi_d)

            # W gates the matmul phases: split each chunk across BOTH HWDGE
            # rings so transfer + completion receipt land ~2x sooner.  x load
            # issue is spread over three rings (sync: chunk-0, scalar: late
            # chunk-0 pieces before its copies start, gpsimd SWDGE: chunk-1)
            # so no single sequencer serializes the load stream.
            W_sb = wpool.tile([P, 2, KMAX * D_H], mm_dt)
            kh = (KMAX // 2) * D_H
            xbuf = xpool.tile([P, 2, NTR], mm_dt)

            def xpiece(eng, c, a, n):
                eng.dma_start(
                    xbuf[:, c, a * P : (a + n) * P],
                    xT_d[c, :, a * P : (a + n) * P],
                )

            nc.scalar.dma_start(W_sb[:, 0, :kh], W_d[:, 0, : KMAX // 2])
            nc.sync.dma_start(W_sb[:, 0, kh:], W_d[:, 0, KMAX // 2 :])
            xpiece(nc.sync, 0, *pieces[0])
            xpiece(nc.gpsimd, 1, *pieces[0])
            nc.scalar.dma_start(W_sb[:, 1, :kh], W_d[:, 1, : KMAX // 2])
            nc.sync.dma_start(W_sb[:, 1, kh:], W_d[:, 1, KMAX // 2 :])
            for i, (a, n) in enumerate(pieces[1:]):
                xpiece(nc.sync if i < 2 else nc.scalar, 0, a, n)
                xpiece(nc.gpsimd, 1, a, n)

            # --- weight-slot index registers.  54 allocatable regs/engine:
            # hoist the first 3 batches (48 regs) to the top so the PE never
            # stalls on a TENSOR_LOAD mid-stream; later batches are emitted
            # two batches ahead of first use.
            n_batches = (NT + TBI - 1) // TBI
            vals = [None] * NT

            def load_batch(b):
                i0 = b * TBI
                ti = min(TBI, NT - i0)
                if static_idx:
                    bv = (0,) * ti
                else:
                    _, bv = nc.values_load_multi_w_load_instructions(
                        wi_sb[0:1, i0 : i0 + ti],
                        engines=(mybir.EngineType.PE,),
                        min_val=0,
                        max_val=(KMAX - 1) * D_H,
                        skip_runtime_bounds_check=True,
                    )
                for j in range(ti):
                    vals[i0 + j] = bv[j]

            for b in range(min(3, n_batches)):
                load_batch(b)
            next_batch = 3

            obuf = opool.tile([P, NT, D_H], out_dt)

            store_from = 0
            for gi, (g0, gs_) in enumerate(groups):
                # prefetch index batches: keep two batches of lead
                while next_batch < n_batches and (
                    next_batch * TBI < g0 + 2 * TBI + gs_
                ):
                    load_batch(next_batch)
                    next_batch += 1

                # Two psum slots (j, j+1) share one 2KB PSUM bank; the bank's
                # has_written clear (start) must be the FIRST write to the
                # bank and the stop the LAST, so: the chunk-0 pass starts on
                # the bank's first slot and the chunk-1 pass stops on its
                # last.  Running all chunk-0 matmuls first lets the first
                # group proceed before W chunk 1 arrives.  The group is split
                # across TWO psum tiles so the ACT and DVE copies are
                # hazard-independent (the Tile dep tracker serializes
                # cross-engine PSUM access per tile).
                h = (gs_ + 1) // 2
                psa = psum_pool.tile([P, 2, D_H], f32)
                psb = psum_pool.tile([P, 2, D_H], f32)
                for c in (0, 1):
                    for j in range(gs_):
                        t = g0 + j
                        pst, jj = (psa, j) if j < h else (psb, j - h)
                        last_in_bank = j == h - 1 or j == gs_ - 1
                        nc.tensor.matmul(
                            pst[:, jj, :],
                            xbuf[:, c, t * P : (t + 1) * P],
                            W_sb[:, c, ds(vals[t], D_H)],
                            start=(c == 0 and jj == 0),
                            stop=(c == 1 and last_in_bank),
                            skip_group_check=True,
                        )
                # split the PSUM->SBUF copy between ACT and DVE
                nc.scalar.copy(obuf[:, g0 : g0 + h], psa[:, :h])
                if gs_ > h:
                    nc.vector.tensor_copy(
                        obuf[:, g0 + h : g0 + gs_], psb[:, : gs_ - h]
                    )
                # store every 8 tiles (2 copy groups) on the GPSIMD ring
                done = g0 + gs_
                if done - store_from >= 8 or gi == len(groups) - 1:
                    nc.gpsimd.dma_start(
                        out_d[:, store_from:done, :],
                        obuf[:, store_from:done],
                    )
                    store_from = done

    nc.compile()
    return nc


def kernel(x=None, cat_ids=None, W=None, b=None, **_unused):
    global last_results
    x = np.asarray(x, np.float32)
    W = np.asarray(W, np.float32)
    N, M, _ = x.shape

    in_maps, scatter, NT, KMAX = _pack(x, cat_ids, W)

    nc = _build(NT, KMAX)

    trace = os.environ.get("CSL_TRACE", "0") == "1"
    kwargs = {}
    if trace:
        kwargs["trace"] = True
        tc_env = os.environ.get("CSL_TRACE_CORES", "")
        if tc_env:
            kwargs["trace_cores"] = [int(c) for c in tc_env.split(",")]
        else:
            kwargs["trace_cores"] = list(range(NCORES))
    res = run_bass_kernel_spmd(
        nc, in_maps, core_ids=list(range(NCORES)), **kwargs
    )
    last_results = res

    out = np.empty((N, M, D_H), np.float32)
    for k in range(NCORES):
        ids, valid = scatter[k]
        # device layout [P, NT, D_H] -> row-major [NT*P, D_H]
        ok = res.results[k]["out"].astype(np.float32, copy=False)
        ok = ok.transpose(1, 0, 2).reshape(NT * SPT, ROWS_PER_SAMPLE, D_H)
        out[ids[valid]] = ok[valid]

    if b is not None:
        b = np.asarray(b, np.float32)
        if np.any(b):
            cat = np.asarray(cat_ids).astype(np.int64).ravel()
            out += b[cat][:, None, :]

    return out


# revision 39
# speedup vs baseline: 1.1600x; 1.1600x over previous
"""Category-specific linear (MoE-routing style) Trainium2 Bass kernel.

Computes out[n] = x[n] @ W[cat_ids[n]] + b[cat_ids[n]] for
x: [N, M, D_IN] f32, cat_ids: [N] int64, W: [C, D_IN, D_H] f32, b: [C, D_H] f32.

Strategy (8-core SPMD, full inputs in / full output out):
  Host: stable-sort samples by category, split into 8 equal shards of
  N/8 samples (perfect load balance).  Within a shard, each category is a
  contiguous run; runs are padded to whole 128-row tiles (8 samples) so the
  device program is fully static.  x rows are pre-transposed on the host
  (fp32 has no DMA-transpose path on TRN2) into [2, 128, NT*128] so the
  contraction dim lands on SBUF partitions.  Each core also gets a small
  deduplicated weight table (its <=KMAX distinct categories) and a per-tile
  weight-slot index.
  Device (v2 layout, tuned from perfetto trace of v1):
    - x lives in ONE resident SBUF buffer, loaded by 6 large piece-DMAs on
      the Sync HWDGE ring (range-based Tile deps give fine-grained readiness).
    - W table in SBUF (chunk-major); per-tile weight slot loaded into PE
      registers (values_load), matmul moving operand dynamically sliced.
    - PSUM groups of 8 tiles; within a group all chunk-0 matmuls run first,
      then all chunk-1 (so the first group only waits on W chunk 0).
    - PSUM->SBUF copies are split per group between Scalar (ACT) and Vector
      (DVE) engines - v1 had all copies on DVE and it was the steady-state
      bottleneck (100% busy).
    - Output staged in one resident SBUF buffer; stores issued per group on
      the GPSIMD (SWDGE) ring so they never contend with load issue or the
      copy engines.
"""

import os
import sys

import numpy as np

for _p in ("/opt/trn_rl_repo",):
    if os.path.isdir(_p) and _p not in sys.path:
        sys.path.insert(0, _p)

import concourse.bass as bass  # noqa: E402
import concourse.mybir as mybir  # noqa: E402
import concourse.tile as tile  # noqa: E402
from concourse import bacc  # noqa: E402
from concourse.bass import ds  # noqa: E402
from concourse.bass_utils import run_bass_kernel_spmd  # noqa: E402

NCORES = 8
P = 128  # SBUF partitions / rows per tile
D_IN = 256  # contraction dim (2 chunks of 128)
D_H = 256  # output dim
ROWS_PER_SAMPLE = 16
SPT = P // ROWS_PER_SAMPLE  # samples per tile = 8
TB = 8  # tile-count quantum (NT is padded to a multiple of this)
TBI = 16  # tiles per index-register load

# filled by kernel() for test harness introspection
last_results = None


def _pack(x, cat_ids, W):
    """Host-side routing: sort, shard, pad, transpose, dedup weights.

    Returns (in_maps, scatter_info, NT, KMAX).
    scatter_info[k] = (sample_ids_per_padded_slot [NT*SPT] int64, valid mask)
    """
    N, M, Din = x.shape
    assert M == ROWS_PER_SAMPLE and Din == D_IN
    assert N % NCORES == 0

    cat = np.asarray(cat_ids).astype(np.int64).ravel()
    order = np.argsort(cat, kind="stable")
    cats_sorted = cat[order]

    # global category runs over the sorted sample list
    bounds = np.flatnonzero(np.diff(cats_sorted)) + 1
    seg_starts = np.concatenate([[0], bounds])
    seg_ends = np.concatenate([bounds, [N]])
    segments = [
        (int(cats_sorted[s]), int(s), int(e))
        for s, e in zip(seg_starts, seg_ends)
    ]

    def pack(T):
        """Greedy-pack category runs into cores of <= T tiles each.

        A run cut mid-category always cuts at an SPT-sample multiple, so
        cuts cost no padding; only each core-local run tail pads to a tile.
        Returns (padded_ids, tile_cats) per core or None if > NCORES cores.
        """
        cores = []
        cur_ids, cur_tcats, used = [], [], 0
        rem = list(segments)
        i = 0

        def close():
            nonlocal cur_ids, cur_tcats, used
            cores.append((cur_ids, cur_tcats))
            cur_ids, cur_tcats, used = [], [], 0

        while i < len(rem):
            c, s, e = rem[i]
            n = e - s
            tiles_need = (n + SPT - 1) // SPT
            avail = T - used
            if avail >= tiles_need:
                npad = (-n) % SPT
                cur_ids.append(order[s:e])
                if npad:
                    cur_ids.append(np.full(npad, -1, np.int64))
                cur_tcats.extend([c] * tiles_need)
                used += tiles_need
                i += 1
            elif avail >= 1:
                take = avail * SPT  # n > take since tiles_need > avail
                cur_ids.append(order[s : s + take])
                cur_tcats.extend([c] * avail)
                used = T
                rem[i] = (c, s + take, e)
                close()
            else:
                close()
            if len(cores) > NCORES:
                return None
        if cur_tcats:
            close()
        if len(cores) > NCORES:
            return None
        while len(cores) < NCORES:
            cores.append(([], []))
        return cores

    lo, hi = (N // NCORES) // SPT, ((N // NCORES) // SPT) * 2 + 16
    while lo < hi:
        mid = (lo + hi) // 2
        if pack(mid) is not None:
            hi = mid
        else:
            lo = mid + 1
    NT = ((lo + 3) // 4) * 4  # multiple of 4
    cores = pack(NT)
    assert cores is not None

    # pad every core to NT tiles (pad with the LAST category so the slot
    # sequence stays monotone / run-contiguous)
    padded_ids = []
    tile_cats = []
    for k in range(NCORES):
        ids_parts, tcats = cores[k]
        n_have = len(tcats)
        extra = NT - n_have
        if extra:
            fill_cat = tcats[-1] if tcats else 0
            tcats = tcats + [fill_cat] * extra
            ids_parts = ids_parts + [np.full(extra * SPT, -1, np.int64)]
        padded_ids.append(
            np.concatenate(ids_parts)
            if ids_parts
            else np.full(NT * SPT, -1, np.int64)
        )
        tile_cats.append(tcats)

    # per-core weight dedup
    uniq_list = []
    for k in range(NCORES):
        seen = dict()
        for c in tile_cats[k]:
            if c not in seen:
                seen[c] = len(seen)
        uniq_list.append(seen)
    KMAX = max(len(u) for u in uniq_list)

    np_in = _np_in_dtype()
    in_maps = []
    scatter = []
    for k in range(NCORES):
        ids = padded_ids[k]
        valid = ids >= 0
        # gather + zero-pad x rows: [NT*SPT, M, Din]
        Xr = np.zeros((NT * SPT, M, Din), np.float32)
        Xr[valid] = x[ids[valid]]
        # transpose to [Din, NT*P] then chunk the contraction dim
        xT = np.ascontiguousarray(
            Xr.reshape(NT * P, Din).T.astype(np_in)
        ).reshape(2, P, NT * P)

        seen = uniq_list[k]
        w_ids = list(seen.keys())
        w_ids += [w_ids[0]] * (KMAX - len(w_ids))
        Wp = W[np.asarray(w_ids, np.int64)]  # [KMAX, Din, D_H]
        Wl = np.ascontiguousarray(
            Wp.reshape(KMAX, 2, P, D_H).transpose(2, 1, 0, 3).astype(np_in)
        )  # [P, 2, KMAX, D_H]

        # pre-scaled by D_H so the device-side dynamic slice lands on a
        # stride-1 dim (no register multiply chain per matmul)
        widx = np.asarray(
            [seen[c] * D_H for c in tile_cats[k]], np.int32
        )[None, :]

        in_maps.append({"xT": xT, "Wl": Wl, "widx": widx})
        scatter.append((ids, valid))

    return in_maps, scatter, NT, KMAX


def _dt_mode():
    return os.environ.get("CSL_DT_MODE", "f16")


def _out_mode():
    return os.environ.get("CSL_OUT_DT", "f16")


def _np_in_dtype():
    import ml_dtypes

    return {
        "f16": np.float16,
        "bf16": ml_dtypes.bfloat16,
        "f32r": np.float32,
        "f32": np.float32,
    }[_dt_mode()]


def _mm_dt():
    return {
        "f16": mybir.dt.float16,
        "bf16": mybir.dt.bfloat16,
        "f32r": mybir.dt.float32r,
        "f32": mybir.dt.float32,
    }[_dt_mode()]


def _groups(NT):
    """Psum/copy group boundaries: 4-tile groups (fine granularity keeps the
    PE dense and copies early; any remainder lands in the last group)."""
    gs = []
    t = 0
    while t < NT:
        g = min(4, NT - t)
        gs.append((t, g))
        t += g
    return gs


def _build(NT, KMAX):
    """Build the SPMD device program for NT tiles and KMAX weight slots."""
    mm_dt = _mm_dt()
    out_dt = mybir.dt.float32 if _out_mode() == "f32" else mybir.dt.float16
    f32 = mybir.dt.float32
    i32 = mybir.dt.int32
    WB = mybir.dt.size(mm_dt)  # bytes per W element
    static_idx = os.environ.get("CSL_STATIC", "0") == "1"

    nc = bacc.Bacc(
        "TRN2",
        target_bir_lowering=False,
        debug=False,
        enable_asserts=False,
        num_devices=NCORES,
    )
    NTR = NT * P
    xT_d = nc.dram_tensor("xT", [2, P, NTR], mm_dt, kind="ExternalInput").ap()
    W_d = nc.dram_tensor("Wl", [P, 2, KMAX, D_H], mm_dt, kind="ExternalInput").ap()
    wi_d = nc.dram_tensor("widx", [1, NT], i32, kind="ExternalInput").ap()
    # partition-major output layout: fully contiguous per-partition stores;
    # the host untransposes when scattering back
    out_d = nc.dram_tensor("out", [P, NT, D_H], out_dt, kind="ExternalOutput").ap()

    groups = _groups(NT)
    # x piece boundaries (in tiles) per chunk: small head then two big pieces
    def _pieces(NT):
        ps = [(0, 4)]
        half = 4 + (NT - 4 + 1) // 2
        ps.append((4, half - 4))
        ps.append((half, NT - half))
        return [(a, n) for a, n in ps if n > 0]

    pieces = _pieces(NT)

    with tile.TileContext(nc) as tc:
        with (
            tc.tile_pool(name="wpool", bufs=1) as wpool,
            tc.tile_pool(name="xpool", bufs=1) as xpool,
            tc.tile_pool(name="opool", bufs=1) as opool,
            tc.tile_pool(name="psum", bufs=4, space="PSUM") as psum_pool,
        ):
            # --- issue order matters: widx first (tiny, unblocks the PE
            # values_load), then W chunk 0, then the first x pieces.
            wi_sb = wpool.tile([1, NT], i32)
            nc.sync.dma_start(wi_sb[:], wi_d)

            # W gates the matmul phases: split each chunk across BOTH HWDGE
            # rings so its transfer (and completion receipt) lands ~2x sooner
            W_sb = wpool.tile([P, 2, KMAX * D_H], mm_dt)
            kh = (KMAX // 2) * D_H
            nc.scalar.dma_start(W_sb[:, 0, :kh], W_d[:, 0, : KMAX // 2])
            nc.sync.dma_start(W_sb[:, 0, kh:], W_d[:, 0, KMAX // 2 :])

            xbuf = xpool.tile([P, 2, NTR], mm_dt)
            # head pieces for both chunks, then W chunk 1, then the rest
            a, n = pieces[0]
            nc.sync.dma_start(
                xbuf[:, 0, a * P : (a + n) * P], xT_d[0, :, a * P : (a + n) * P]
            )
            nc.scalar.dma_start(W_sb[:, 1, :kh], W_d[:, 1, : KMAX // 2])
            nc.sync.dma_start(
                xbuf[:, 1, a * P : (a + n) * P], xT_d[1, :, a * P : (a + n) * P]
            )
            nc.sync.dma_start(W_sb[:, 1, kh:], W_d[:, 1, KMAX // 2 :])
            for a, n in pieces[1:]:
                nc.sync.dma_start(
                    xbuf[:, 0, a * P : (a + n) * P],
                    xT_d[0, :, a * P : (a + n) * P],
                )
                nc.sync.dma_start(
                    xbuf[:, 1, a * P : (a + n) * P],
                    xT_d[1, :, a * P : (a + n) * P],
                )

            # --- weight-slot index registers.  54 allocatable regs/engine:
            # hoist the first 3 batches (48 regs) to the top so the PE never
            # stalls on a TENSOR_LOAD mid-stream; later batches are emitted
            # two batches ahead of first use.
            n_batches = (NT + TBI - 1) // TBI
            vals = [None] * NT

            def load_batch(b):
                i0 = b * TBI
                ti = min(TBI, NT - i0)
                if static_idx:
                    bv = (0,) * ti
                else:
                    _, bv = nc.values_load_multi_w_load_instructions(
                        wi_sb[0:1, i0 : i0 + ti],
                        engines=(mybir.EngineType.PE,),
                        min_val=0,
                        max_val=(KMAX - 1) * D_H,
                        skip_runtime_bounds_check=True,
                    )
                for j in range(ti):
                    vals[i0 + j] = bv[j]

            for b in range(min(3, n_batches)):
                load_batch(b)
            next_batch = 3

            obuf = opool.tile([P, NT, D_H], out_dt)

            store_from = 0
            for gi, (g0, gs_) in enumerate(groups):
                # prefetch index batches: keep two batches of lead
                while next_batch < n_batches and (
                    next_batch * TBI < g0 + 2 * TBI + gs_
                ):
                    load_batch(next_batch)
                    next_batch += 1

                # Two psum slots (j, j+1) share one 2KB PSUM bank; the bank's
                # has_written clear (start) must be the FIRST write to the
                # bank and the stop the LAST, so: the chunk-0 pass starts on
                # the bank's first slot and the chunk-1 pass stops on its
                # last.  Running all chunk-0 matmuls first lets the first
                # group proceed before W chunk 1 arrives.  The group is split
                # across TWO psum tiles so the ACT and DVE copies are
                # hazard-independent (the Tile dep tracker serializes
                # cross-engine PSUM access per tile).
                h = (gs_ + 1) // 2
                psa = psum_pool.tile([P, 2, D_H], f32)
                psb = psum_pool.tile([P, 2, D_H], f32)
                for c in (0, 1):
                    for j in range(gs_):
                        t = g0 + j
                        pst, jj = (psa, j) if j < h else (psb, j - h)
                        last_in_bank = j == h - 1 or j == gs_ - 1
                        nc.tensor.matmul(
                            pst[:, jj, :],
                            xbuf[:, c, t * P : (t + 1) * P],
                            W_sb[:, c, ds(vals[t], D_H)],
                            start=(c == 0 and jj == 0),
                            stop=(c == 1 and last_in_bank),
                            skip_group_check=True,
                        )
                # split the PSUM->SBUF copy between ACT and DVE
                nc.scalar.copy(obuf[:, g0 : g0 + h], psa[:, :h])
                if gs_ > h:
                    nc.vector.tensor_copy(
                        obuf[:, g0 + h : g0 + gs_], psb[:, : gs_ - h]
                    )
                # store every 8 tiles (2 copy groups) on the GPSIMD ring
                done = g0 + gs_
                if done - store_from >= 8 or gi == len(groups) - 1:
                    nc.gpsimd.dma_start(
                        out_d[:, store_from:done, :],
                        obuf[:, store_from:done],
                    )
                    store_from = done

    nc.compile()
    return nc


def kernel(x=None, cat_ids=None, W=None, b=None, **_unused):
    global last_results
    x = np.asarray(x, np.float32)
    W = np.asarray(W, np.float32)
    N, M, _ = x.shape

    in_maps, scatter, NT, KMAX = _pack(x, cat_ids, W)

    nc = _build(NT, KMAX)

    trace = os.environ.get("CSL_TRACE", "0") == "1"
    kwargs = {}
    if trace:
        kwargs["trace"] = True
        tc_env = os.environ.get("CSL_TRACE_CORES", "")
        if tc_env:
            kwargs["trace_cores"] = [int(c) for c in tc_env.split(",")]
        else:
            kwargs["trace_cores"] = list(range(NCORES))
    res = run_bass_kernel_spmd(
        nc, in_maps, core_ids=list(range(NCORES)), **kwargs
    )
    last_results = res

    out = np.empty((N, M, D_H), np.float32)
    for k in range(NCORES):
        ids, valid = scatter[k]
        # device layout [P, NT, D_H] -> row-major [NT*P, D_H]
        ok = res.results[k]["out"].astype(np.float32, copy=False)
        ok = ok.transpose(1, 0, 2).reshape(NT * SPT, ROWS_PER_SAMPLE, D_H)
        out[ids[valid]] = ok[valid]

    if b is not None:
        b = np.asarray(b, np.float32)
        if np.any(b):
            cat = np.asarray(cat_ids).astype(np.int64).ravel()
            out += b[cat][:, None, :]

    return out
